# revision 51
# baseline (speedup 1.0000x reference)
"""Trainium2 Bass kernel for nn_MixedAttnHeadEmbed_82076825027210.

Computes, per batch element:
    out = sum over h in {4, 8, 12} of CausalAttention(Q_mix_h, K_mix_h, V_mix_h)
where Q/K/V_mix_h are weighted mixtures (9 scalar weights) of head-sliced
views of x's q/k/v channel groups, zero-padded per head to hd = 768/h.

Sharding: data-parallel over batch B=8 across the 8 NeuronCores (one batch
element per core); the 9 mixture weights are baked into the compiled program
as immediates.

Per-core plan (T=1024 tokens, bf16 compute):
  1. SWDGE cast-DMA loads x f32->bf16 once (9.4MB HBM; with the 3.1MB
     output write that is all the HBM traffic -- the baseline moved 52MB).
  2. Mixing: DVE builds mixed Q/K naturals (scalar_tensor_tensor; Pool
     lacks that opcode and takes ts+tt pairs for the run-ahead configs'
     K/V).  V_aug [t, h, hd+1] carries a ones-column per head (softmax
     denominator).  Emission is interleaved per token tile with the loads,
     and config ci+1's mixing is emitted inside config ci's attention
     chunks so DVE/Pool run ahead (engines execute in per-engine emission
     order).  Config order (12, 8, 4): the most exp-heavy config's ACT
     phase hides the serial x-load + transpose DMA stream.
  3. Q/K naturals transpose to Q^T/K^T [d, T] bf16 via SBUF->SBUF
     DMA-transpose (XBAR) -- no DRAM bounce, no PE transposes.  Heads are
     zero-padded to whole 128-row slabs: all S^T contraction chunks start
     at partition 0 with 96/128 rows (a single sub-128-row matmul group at
     an unaligned PSUM offset faults the device).
  4. Attention per config in 256-wide query chunks: S^T = K Q^T blockwise
     on PE (causal blocks only, diagonal masked by an extra matmul with a
     strict-triangular constant), exp on ACT over 4-head PSUM slabs
     (softmax scale folded in; max-subtraction skipped: |S*scale| << 1),
     giving P^T bf16 in SBUF directly.
  5. PV natural: Y[q, e] accumulates matmul(lhsT=P^T, rhs=V_aug) over key
     blocks in PSUM f32, heads packed h/2 per bank; PSUM accumulation
     groups are bank-granular (one start / one stop per bank).  The
     ones-column yields the per-head denominator.  No Y transposes.
  6. Normalize on DVE straight from PSUM: strided reciprocal of the
     denominators, then one scalar_tensor_tensor per head accumulating
     into the bf16 output accumulator.
  7. Per-qtile SWDGE cast-DMA (bf16->f32) writes the result, overlapped
     with the last config's attention.
"""

import math
import os

import numpy as np

# bisect aid: 1=loads+mix+transpose, 2=+S^T/exp, 3=+PV, 4=full
_DBG = float(os.environ.get("KDBG", "4"))
# bisect aid: which ORDER positions run attention (default all)
_DBG_POS = set(int(c) for c in os.environ.get("KPOS", "012"))
_DBG_NODIAG = os.environ.get("KNODIAG", "") == "1"
_DBG_NSG = int(os.environ.get("KSG", "99"))
_DBG_NCH = int(os.environ.get("KCH", "99"))

import concourse.bass as bass
import concourse.bacc as bacc
import concourse.tile as tile
from concourse import mybir
from concourse.bass_utils import run_bass_kernel_spmd

F32 = mybir.dt.float32
BF16 = mybir.dt.bfloat16
ALU = mybir.AluOpType
ACTF = mybir.ActivationFunctionType

T = 1024
NT = 8  # token tiles of 128
E = 768
CIN = 3 * E
N_HEAD_LIST = (4, 8, 12)
EMBED_DIM_LIST = (384, 576, 768)
N_CORES = 8
MASK_NEG = -3000.0  # pre-scale additive mask; exp(scale*-3000) == 0 in f32
W_Q = 256  # query-chunk width (2 qtiles)
NCHUNK = T // W_Q


def _phead(h):
    """Per-head padded width in the transposed Q/K layout.  Every head is
    zero-padded to a whole number of 128-row slabs: matmul contraction
    chunks then always start at partition 0 with >=96 rows (single sub-128
    row matmul groups at unaligned PSUM offsets fault on HW)."""
    return 256 if h == 4 else 128


def _nslab(h):
    return h * _phead(h) // 128


def _dchunks(h):
    """Per head: list of (slab, base, size) contraction ranges in the
    padded transposed layout; 128-row chunks (96 for h=8, pads unused)."""
    if h == 4:
        return [[(2 * j, 0, 128), (2 * j + 1, 0, 128)] for j in range(h)]
    if h == 8:
        return [[(j, 0, 96)] for j in range(h)]
    return [[(j, 0, 128)] for j in range(h)]


def _ypack(h):
    """Per head: (bank, col offset) in the [128, 2, 512] f32 Y tile; each
    head occupies hd+1 cols (data + denominator) fully inside one bank,
    h/2 heads per bank so denominator APs are uniform."""
    hd = E // h
    w = hd + 1
    per_bank = h // 2
    assert per_bank * w <= 512
    return [(j // per_bank, (j % per_bank) * w) for j in range(h)]


def _build_program(W):
    """W: numpy [9] f32 mixture weights. Returns compiled Bacc program."""
    nc = bacc.Bacc(
        "TRN2", target_bir_lowering=False, debug=False, num_devices=N_CORES
    )
    x_in = nc.dram_tensor("x", [T, CIN], F32, kind="ExternalInput").ap()
    out_d = nc.dram_tensor("out", [T, E], F32, kind="ExternalOutput").ap()
    with tile.TileContext(nc) as tc:
        _emit(tc, x_in, out_d, W)
    nc.compile()
    return nc


def _emit(tc, x_in, out_d, W):
    nc = tc.nc
    with (
        tc.tile_pool(name="consts", bufs=1) as consts,
        tc.tile_pool(name="xin", bufs=1) as xpool,
        tc.tile_pool(name="nat", bufs=2) as nat_pool,
        tc.tile_pool(name="qkt", bufs=2) as qkt_pool,
        tc.tile_pool(name="vaug", bufs=2) as vaug_pool,
        tc.tile_pool(name="pt", bufs=10) as pt_pool,
        tc.tile_pool(name="mixtmp", bufs=2) as tmp_pool,
        tc.tile_pool(name="small", bufs=8) as small_pool,
        tc.tile_pool(name="oacc", bufs=1) as oacc_pool,
        tc.tile_pool(name="stage", bufs=2, space="PSUM") as stage_pool,
        tc.tile_pool(name="ypsum", bufs=2, space="PSUM") as ypsum_pool,
    ):
        # ---- constants: strict-upper and MASK_NEG*I for diagonal masking --
        ustrict = consts.tile([128, 128], BF16)
        nc.gpsimd.memset(ustrict, 1.0)
        nc.gpsimd.affine_select(
            out=ustrict, in_=ustrict, compare_op=ALU.is_gt, fill=0.0,
            base=0, pattern=[[1, 128]], channel_multiplier=-1,
        )
        negi = consts.tile([128, 128], BF16)
        nc.gpsimd.memset(negi, 0.0)
        nc.gpsimd.affine_select(
            out=negi, in_=negi, compare_op=ALU.not_equal, fill=MASK_NEG,
            base=0, pattern=[[-1, 128]], channel_multiplier=1,
        )

        xqk = xpool.tile([128, NT, 2 * E], BF16)  # q/k channels, cast once
        xv = xpool.tile([128, NT, E], BF16)  # v channels, cast once
        oacc = oacc_pool.tile([128, NT, E], BF16)
        outv = out_d.rearrange("(a p) c -> p a c", p=128)

        # ---- per-config mixing state -----------------------------------
        cfg = {}

        def prep(ci):
            h = N_HEAD_LIST[ci]
            hd = E // h
            ph = _phead(h)
            S = _nslab(h)
            # 4 round-robin per-config nat buffers; pads zeroed once (they
            # are never written again, so stay zero across reuses)
            nats = [
                nat_pool.tile(
                    [128, S * 128], BF16, tag=f"nat{ci}", name=f"nat{ci}_{i}"
                )
                for i in range(4)
            ]
            for nt_ in nats:
                nc.vector.memset(
                    nt_.rearrange("p (h d) -> p h d", h=h)[:, :, hd:ph], 0.0
                )
            # h=4 allocates from the (larger) h=12 ring: h=12's tiles are
            # dead early, so h=4's transposes don't WAR-wait on h=8's
            # still-active tiles in the qkt8 ring.
            big = h != 8
            tag = "qkt12" if big else "qkt8"
            shape = [128, 12 if big else 8, T]
            qt = qkt_pool.tile(shape, BF16, tag=tag, name=f"qt{ci}")
            kt = qkt_pool.tile(shape, BF16, tag=tag, name=f"kt{ci}")
            vflat = vaug_pool.tile([128, NT, 780], BF16, tag="vaug")
            nc.gpsimd.memset(
                vflat[:, :, 0 : h * (hd + 1)].rearrange(
                    "p t (h w) -> p t h w", h=h
                )[:, :, :, hd : hd + 1],
                1.0,
            )
            cfg[ci] = dict(nats=nats, qt=qt, kt=kt, vflat=vflat)

        def vview(ci, t):
            h = N_HEAD_LIST[ci]
            hd = E // h
            return cfg[ci]["vflat"][:, t, 0 : h * (hd + 1)].rearrange(
                "p (h w) -> p h w", h=h
            )

        def mix_tile(ci, t, load=False):
            """Emit loads (cfg0 only), Q/K mixing + transposes (DVE+HWDGE)
            and V mixing (Pool) for token tile t of config ci."""
            h = N_HEAD_LIST[ci]
            hd = E // h
            S = _nslab(h)
            w768 = float(W[3 * ci + 2])
            w576 = float(W[3 * ci + 1])
            w384 = float(W[3 * ci + 0])
            if load:
                for half in range(2):  # q then k: finer grain unblocks DVE
                    nc.gpsimd.dma_start(
                        out=xqk[:, t, half * E : (half + 1) * E],
                        in_=x_in[
                            t * 128 : (t + 1) * 128, half * E : (half + 1) * E
                        ],
                    )
            def mix(eng, out_ap, in_ap, tmp_ap):
                """out = w768*in(768) + w576*in(576) + w384*in(384), head-
                sliced.  DVE uses fused scalar_tensor_tensor; Pool lacks
                that opcode so it takes a ts+tt pair via tmp."""
                eng.tensor_scalar(
                    out_ap(768), in_ap(768), w768, None, ALU.mult
                )
                for e, w in ((576, w576), (384, w384)):
                    if eng is nc.vector:
                        eng.scalar_tensor_tensor(
                            out=out_ap(e), in0=in_ap(e), scalar=w,
                            in1=out_ap(e), op0=ALU.mult, op1=ALU.add,
                        )
                    else:
                        eng.tensor_scalar(
                            tmp_ap(e), in_ap(e), w, None, ALU.mult
                        )
                        eng.tensor_tensor(
                            out_ap(e), tmp_ap(e), out_ap(e), ALU.add
                        )

            for tensor_idx, tl in (
                (0, cfg[ci]["qt"]),
                (1, cfg[ci]["kt"]),
            ):
                base = tensor_idx * E
                nat = cfg[ci]["nats"][(2 * t + tensor_idx) % 4]
                # K-mixing of the run-ahead configs goes to Pool to unload
                # DVE; the first (prologue) config keeps K on DVE.
                eng = nc.vector if (tensor_idx == 0 or load or ci == 0) else nc.gpsimd
                tmp = (
                    None
                    if eng is nc.vector
                    else tmp_pool.tile([128, 768], BF16, tag="mixtmp")
                )

                def out_ap(e):
                    return nat.rearrange("p (h d) -> p h d", h=h)[
                        :, :, 0 : e // h
                    ]

                def in_ap(e):
                    return xqk[:, t, base : base + e].rearrange(
                        "p (h d) -> p h d", h=h
                    )

                def tmp_ap(e):
                    return tmp[:, 0:e].rearrange("p (h d) -> p h d", h=h)

                mix(eng, out_ap, in_ap, tmp_ap)
                nc.sync.dma_start(
                    out=tl[:, 0:S, t * 128 : (t + 1) * 128],
                    in_=nat[:, :],
                    transpose=True,
                )
            if load:
                nc.gpsimd.dma_start(
                    out=xv[:, t, :],
                    in_=x_in[t * 128 : (t + 1) * 128, 2 * E : CIN],
                )
            # V mixing (reads f32 xv at no extra cost).  First config on
            # DVE: Pool's in-order queue is saturated by the x loads then,
            # and PV would stall on V tiles queued behind them.
            veng = nc.vector if load else nc.gpsimd
            vtmp = (
                None
                if veng is nc.vector
                else tmp_pool.tile([128, 768], BF16, tag="mixtmp")
            )
            vv = vview(ci, t)
            mix(
                veng,
                lambda e: vv[:, :, 0 : e // h],
                lambda e: xv[:, t, 0:e].rearrange("p (h d) -> p h d", h=h),
                lambda e: vtmp[:, 0:e].rearrange("p (h d) -> p h d", h=h),
            )

        # ---- prologue: first config's loads + mixing, tile by tile ------
        # Config order: most ACT-heavy (h=12) first so its exp phase hides
        # the DMA trickle (loads + transposes share one DMA resource).
        ORDER = (2, 1, 0)
        prep(ORDER[0])
        for t in range(NT):
            mix_tile(ORDER[0], t, load=True)

        # ---- attention per config ---------------------------------------
        for pos, ci in enumerate(ORDER):
            h = N_HEAD_LIST[ci]
            hd = E // h
            scale = 1.0 / math.sqrt(hd)
            dchunks = _dchunks(h)
            ypack = _ypack(h)
            nb0 = sum(1 for b, _ in ypack if b == 0)
            nsg = (h + 3) // 4
            qt, kt = cfg[ci]["qt"], cfg[ci]["kt"]
            if pos + 1 < 3:
                prep(ORDER[pos + 1])

            for c in range(NCHUNK):
                q0 = 2 * c
                if _DBG < 2 or pos not in _DBG_POS or c >= _DBG_NCH:
                    if pos + 1 < 3:
                        mix_tile(ORDER[pos + 1], 2 * c)
                        mix_tile(ORDER[pos + 1], 2 * c + 1)
                    continue
                kbs = list(range(0, q0 + 2))
                ys = [
                    ypsum_pool.tile(
                        [128, 2, 512], F32, tag="y", name=f"y{tq}"
                    )
                    for tq in (q0, q0 + 1)
                ]
                for wave0 in range(0, len(kbs), 2):
                    wave = kbs[wave0 : wave0 + 2]
                    pts = {}
                    for kb in wave:
                        lo = max(0, kb * 128 - c * W_Q)
                        for sg in range(min(nsg, _DBG_NSG)):
                            stage = stage_pool.tile([128, 4, W_Q], F32)
                            for j in range(4):
                                head = sg * 4 + j
                                diag = (
                                    kb * 128 >= c * W_Q
                                ) and not _DBG_NODIAG
                                chunks = dchunks[head]
                                n_mm = len(chunks) + (1 if diag else 0)
                                for mi, (slab, cb, cs) in enumerate(chunks):
                                    nc.tensor.matmul(
                                        out=stage[:, j, lo:W_Q],
                                        lhsT=kt[
                                            cb : cb + cs,
                                            slab,
                                            kb * 128 : (kb + 1) * 128,
                                        ],
                                        rhs=qt[
                                            cb : cb + cs,
                                            slab,
                                            c * W_Q + lo : (c + 1) * W_Q,
                                        ],
                                        start=(mi == 0),
                                        stop=(mi == n_mm - 1),
                                    )
                                if diag:
                                    nc.tensor.matmul(
                                        out=stage[:, j, lo : lo + 128],
                                        lhsT=ustrict[:, :],
                                        rhs=negi[:, :],
                                        start=False,
                                        stop=True,
                                    )
                            pt = pt_pool.tile([128, 4, W_Q], BF16, tag="pt")
                            if _DBG >= 2.5:
                                nc.scalar.activation(
                                    out=pt[:, :, lo:W_Q],
                                    in_=stage[:, :, lo:W_Q],
                                    func=ACTF.Exp,
                                    scale=scale,
                                )
                            else:
                                nc.vector.tensor_copy(
                                    pt[:, :, lo:W_Q], stage[:, :, lo:W_Q]
                                )
                            pts[(kb, sg)] = pt
                    # PV for this wave into both resident qtiles.  PSUM
                    # groups are bank-granular: one start (kb=0, head 0)
                    # and one stop (kb=tq, last head) per y tile.
                    for ti, tq in enumerate((q0, q0 + 1)):
                        if _DBG < 3:
                            continue
                        for kb in wave:
                            if kb > tq:
                                continue
                            qoff = tq * 128 - c * W_Q
                            for head in range(h):
                                bank, off = ypack[head]
                                first = head in (0, nb0)
                                last = head in (nb0 - 1, h - 1)
                                nc.tensor.matmul(
                                    out=ys[ti][:, bank, off : off + hd + 1],
                                    lhsT=pts[(kb, head // 4)][
                                        :, head % 4, qoff : qoff + 128
                                    ],
                                    rhs=vview(ci, kb)[:, head, :],
                                    start=(kb == 0 and first),
                                    stop=(kb == tq and last),
                                )
                # ---- normalize + accumulate into oacc (DVE, from PSUM) --
                for ti, tq in enumerate((q0, q0 + 1)):
                    if _DBG < 4:
                        continue
                    y = ys[ti]
                    w = hd + 1
                    rec = small_pool.tile([128, 16], F32, tag="rec")
                    nc.vector.reciprocal(
                        rec[:, 0:h].rearrange("p (b n) -> p b n", b=2),
                        y[:, :, hd : hd + (nb0 - 1) * w + 1 : w],
                    )
                    for head in range(h):
                        bank, off = ypack[head]
                        dst = oacc[:, tq, head * hd : (head + 1) * hd]
                        if pos == 0:
                            nc.vector.tensor_scalar(
                                dst,
                                y[:, bank, off : off + hd],
                                rec[:, head : head + 1],
                                None,
                                ALU.mult,
                            )
                        else:
                            nc.vector.scalar_tensor_tensor(
                                out=dst,
                                in0=y[:, bank, off : off + hd],
                                scalar=rec[:, head : head + 1],
                                in1=dst,
                                op0=ALU.mult,
                                op1=ALU.add,
                            )
                    if pos == 2:
                        # result for this qtile is final: write it out
                        nc.gpsimd.dma_start(
                            out=outv[:, tq, :], in_=oacc[:, tq, :]
                        )
                # run-ahead: next config's mixing, after this chunk's
                # normalize so evictions lead the DVE/Pool queues (PSUM y
                # buffers recycle without waiting on mixing)
                if pos + 1 < 3:
                    mix_tile(ORDER[pos + 1], 2 * c)
                    mix_tile(ORDER[pos + 1], 2 * c + 1)


_PROGRAM_CACHE = {}


def _get_program(W):
    key = np.asarray(W, dtype=np.float32).tobytes()
    if key not in _PROGRAM_CACHE:
        _PROGRAM_CACHE[key] = _build_program(np.asarray(W, dtype=np.float32))
    return _PROGRAM_CACHE[key]


def kernel(x, weights):
    """x: [8, 1024, 2304] f32; weights: [9] f32 -> [8, 1024, 768] f32."""
    x = np.asarray(x, dtype=np.float32)
    weights = np.asarray(weights, dtype=np.float32)
    assert x.shape == (N_CORES, T, CIN), x.shape
    nc = _get_program(weights)
    in_maps = [{"x": np.ascontiguousarray(x[c])} for c in range(N_CORES)]
    res = run_bass_kernel_spmd(nc, in_maps, list(range(N_CORES)))
    return np.stack([res.results[c]["out"] for c in range(N_CORES)], axis=0)
